# revision 56
# baseline (speedup 1.0000x reference)
"""Block-sparse attention (ViT-style block-causal) on 8 Trainium2 NeuronCores.

Strategy: data-parallel over batch (4 batches per core), SPMD, no collectives.

Math (per batch, tokens pre-permuted on host into block-sorted order so the
mask is block-causal with 16-token blocks):
  - qkv projection computed directly in transposed layouts:
      QT/KT [o, t] = wqkT.T @ xT   (o on partitions, 2 heads per 128-tile)
      V     [t, o] = xT.T  @ wvT   (natural layout, + ones column appended)
  - S^T[k, q] = KT.T @ QT per head (contraction over head dim = 64)
  - P^T = exp(scale * S^T)  (no max-subtraction; scores bounded ~|6.5|)
    multiplied by a 16-granular staircase 0/1 mask on the diagonal 128-tile
  - A^T_unnorm[d, q] (+ denom row) = V_aug.T @ P^T   (augmented-ones trick:
    row 64 of the output accumulates sum_k P^T = softmax denominator)
  - denominators collected per batch -> reciprocal -> broadcast across
    partitions via a K=12 selector matmul -> A^T = A^T_unnorm * recip
  - y[t, o] = A^T.T @ wpT  (A^T layout feeds proj directly, transpose-free)

All matmul operands bf16 (fp32 PSUM accumulation); output fp32.

Key optimizations over the straightforward schedule:
  - The two heads of a pair compute S^T concurrently on hardware: their
    K=64 matmuls are row-tiled onto PE row groups 0:64 / 64:128 via
    tile_position (the 128x128 array is 16 independent 32x32 subarrays),
    writing adjacent PSUM banks of one [128, 2, 512] pair tile.
  - One Exp and one staircase-mask multiply cover both heads of a pair
    through 3D [*, 2, *] access patterns, halving the ACT/Pool per-
    instruction access overhead (185ns/instr on ACT).
  - Deep software pipelining: batch b+1's qkv and batch b-1's proj are
    chopped into single-accumulation-group steps and threaded into batch
    b's attention at every stall point (after each S chunk / AV / norm),
    paced uniformly against the available slots; unconsumed proj steps
    carry forward to the (filler-starved) later batches.
  - Pair normalization is split: reciprocal+cast run right after the
    pair's AV (off the critical path); the recip-broadcast matmul and
    normalize multiplies run 3 pairs later.
  - 8 warm-up matmuls on a zeroed tile fill the initial weight-DMA wait
    and bring the PE HAM clock gate to 2.4 GHz before real work.
  - Engine assignment: PE matmuls; ACT exp + qkv copies (exp/copy share
    one activation table set - no reloads); DVE PSUM copies + normalize
    + reciprocal; Pool (GpSimd) all staircase masks; y DMAs column-split
    so the last store overlaps the final proj copies.
"""

import numpy as np
import ml_dtypes

B, N, C, H = 32, 576, 768, 12
HD = C // H                      # 64
CORES = 8
BL = B // CORES                  # 4 batches per core
T = BL * N                       # 2304 tokens per core
GRID, BS = 24, 4
SCALE = HD ** -0.5
CA = C // 128                    # 6 contraction tiles
BF16 = ml_dtypes.bfloat16

# q-chunks of the S^T matmul for each key tile kk: (q_offset, width)
S_CHUNKS = {
    0: [(0, 512), (512, 64)],
    1: [(128, 448)],
    2: [(256, 320)],
    3: [(384, 192)],
    4: [(512, 64)],
}
KSZ = [128, 128, 128, 128, 64]   # key-tile sizes (576 = 4*128 + 64)
TSZ = KSZ                        # token-tile sizes within a batch

FP8 = ml_dtypes.float8_e4m3        # TRN FP8_EXP4: max normal +-240
XS = 16.0                          # x pre-scale before fp8 cast
WS = 256.0                         # wqk pre-scale before fp8 cast
FP8_QS = XS * WS                   # Q/K carry this extra factor in fp8 mode
USE_FP8 = False                    # fp8 QK noise ~5% L2 - over the 2e-2 gate

TRACE = False
LAST_EXEC_NS = None
LAST_TRACE_PATH = None

_prog_cache = {}


def _block_perm():
    r = np.arange(GRID) // BS
    bi = (r[:, None] * (GRID // BS) + r[None, :]).reshape(-1)
    return np.argsort(bi, kind="stable")


def _build_program(have_qkb=False, debug_taps=False):
    from contextlib import ExitStack

    import concourse.mybir as mybir
    import concourse.tile as tile
    from concourse import bacc

    dt = mybir.dt
    f32 = dt.float32
    bf16 = dt.bfloat16
    fp8 = dt.float8e4
    mult = mybir.AluOpType.mult
    use_fp8 = USE_FP8 and not have_qkb
    exp_scale = SCALE / (FP8_QS * FP8_QS) if use_fp8 else SCALE

    nc = bacc.Bacc("TRN2", target_bir_lowering=False, debug=False,
                   num_devices=CORES)

    xT_d = nc.dram_tensor("xT", [BL, 128, CA, N], bf16, kind="ExternalInput").ap()
    if use_fp8:
        # contraction dim pair-interleaved for DoubleRow: [pair, 128, plane, .]
        xT8_d = nc.dram_tensor("xT8", [BL, 3, 128, 2, N], fp8,
                               kind="ExternalInput").ap()
        wqk8_d = nc.dram_tensor("wqk8", [3, 128, 2, 2 * C], fp8,
                                kind="ExternalInput").ap()
    else:
        wqk_d = nc.dram_tensor("wqkT", [2, 6, 128, CA, 128], bf16,
                               kind="ExternalInput").ap()
    wv_d = nc.dram_tensor("wvT", [128, CA, C], bf16, kind="ExternalInput").ap()
    wp_d = nc.dram_tensor("wpT", [128, CA, C], bf16, kind="ExternalInput").ap()
    mask_d = nc.dram_tensor("maskT", [128, 2, 128], bf16, kind="ExternalInput").ap()
    y_d = nc.dram_tensor("y", [T, C], bf16, kind="ExternalOutput").ap()
    if have_qkb:
        qkb_d = nc.dram_tensor("qkb", [128, 12], f32, kind="ExternalInput").ap()
    if debug_taps:
        qkt_tap = nc.dram_tensor("qkt_tap", [128, 12, N], bf16,
                                 kind="ExternalOutput").ap()
        pt_tap = nc.dram_tensor("pt_tap", [CA, 2, 128, 2, N], bf16,
                                kind="ExternalOutput").ap()
        atu_tap = nc.dram_tensor("atu_tap", [H, HD + 1, N], f32,
                                 kind="ExternalOutput").ap()

    with tile.TileContext(nc) as tc, ExitStack() as ctx:
        consts = ctx.enter_context(tc.tile_pool(name="consts", bufs=1))
        xt_pool = ctx.enter_context(tc.tile_pool(name="xt", bufs=2))
        qkt_pool = ctx.enter_context(tc.tile_pool(name="qkt", bufs=2))
        pt_pool = ctx.enter_context(tc.tile_pool(name="pt", bufs=5))
        at_pool = ctx.enter_context(tc.tile_pool(name="at", bufs=2))
        d_pool = ctx.enter_context(tc.tile_pool(name="d", bufs=4))
        y_pool = ctx.enter_context(tc.tile_pool(name="y", bufs=6))

        ps_mm = ctx.enter_context(tc.tile_pool(name="ps_mm", bufs=2, space="PSUM"))
        ps_s = ctx.enter_context(tc.tile_pool(name="ps_s", bufs=2, space="PSUM"))
        ps_av = ctx.enter_context(tc.tile_pool(name="ps_av", bufs=2, space="PSUM"))

        # ---- constants / weights (per-contraction-tile tiles so the first
        # matmuls only wait on the first DMA) ----
        if use_fp8:
            wqk8 = [consts.tile([128, 2, 2 * C], fp8, tag=f"wqk8_{p}",
                                name=f"wqk8_{p}") for p in range(3)]
        else:
            # split by output-channel halves so the first 6 ot groups only
            # wait on half the weight bytes; contraction tiles joined so
            # loads go out in 2-tile DMAs (halves HWDGE issue serialization)
            wqk = [consts.tile([128, 6, CA, 128], bf16, tag=f"wqkh{hf}",
                               name=f"wqkh{hf}") for hf in range(2)]
        wv = consts.tile([128, CA, C], bf16, tag="wv", name="wv")
        wp = consts.tile([128, CA, C], bf16, tag="wp", name="wp")
        maskT2 = consts.tile([128, 2, 128], bf16, tag="mask")
        # persistent double-buffered V (by batch parity): [token, kk, head,
        # 2*HD] where columns 0:HD are constant 1.0 and HD:2*HD hold V, so
        # every AV matmul (lhsT [ksz, 128], full PE width) replicates the
        # softmax denominator sum_k P^T across output partitions 0:64 (base
        # 0, as the DVE custom reciprocal requires) with A^T_unnorm on
        # partitions 64:128. Reciprocal and normalize both read the PSUM
        # directly - no A^T staging copies, no cross-partition broadcast,
        # no E-matmul, no denominator DMA.
        warm = consts.tile([128, 512], bf16, tag="warm")
        nc.vector.memset(warm, 0.0)
        v_tiles = [consts.tile([128, 5, H, 2 * HD], bf16, tag=f"v{i}",
                               name=f"v{i}") for i in range(2)]
        for vt in v_tiles:
            for kk in range(5):
                nc.vector.memset(vt[:, kk, :, 0:HD], 1.0)

        _dma_engines = [nc.sync]

        _q3 = [nc.sync, nc.gpsimd, nc.scalar]

        def emit_weight_loads():
            # wp rides each queue behind the critical startup set
            for i in range(3):
                _q3[i].dma_start(
                    out=wp[:, 2 * i:2 * i + 2, :],
                    in_=wp_d[:, 2 * i:2 * i + 2, :])
        if have_qkb:
            qkb = consts.tile([128, 12], f32, tag="qkb")
            nc.sync.dma_start(out=qkb, in_=qkb_d[:, :])

        def emit_xt(b, eng=None):
            xt = xt_pool.tile([128, CA, N], bf16, tag="xt", name="xt")
            for i in range(3):
                e = eng or (nc.sync if i % 2 == 0 else nc.gpsimd)
                e.dma_start(
                    out=xt[:, 2 * i:2 * i + 2, :],
                    in_=xT_d[b, :, 2 * i:2 * i + 2, :])
            return xt, None

        def _load_wqk_oc(hf, oc, eng):
            # one output-channel tile (the weights of qk group ot=6*hf+oc),
            # all contraction tiles: 197KB, so weight chunks land in the
            # order the qk groups consume them (startup is DMA-BW-bound at
            # ~200GB/s aggregate; per-queue ~70GB/s when all three run).
            eng.dma_start(out=wqk[hf][:, oc], in_=wqk_d[hf, oc])

        def emit_startup_loads():
            """Batch 0's x tiles + the wqk oc-tiles in consumption order,
            spread across three DMA queues (sync/gpsimd/scalar)."""
            xt = xt_pool.tile([128, CA, N], bf16, tag="xt", name="xt")
            # queue i: x chunk first (ramp rhs), then its oc-tiles
            _load_wqk_oc(0, 0, nc.sync)
            nc.scalar.dma_start(out=maskT2, in_=mask_d[:, :, :])
            for i in range(3):
                _q3[i].dma_start(out=xt[:, 2 * i:2 * i + 2, :],
                                 in_=xT_d[0, :, 2 * i:2 * i + 2, :])
            _load_wqk_oc(1, 0, nc.sync)       # ot6 (ramp pair 0 K)
            _load_wqk_oc(0, 1, nc.gpsimd)     # ot1
            _load_wqk_oc(1, 1, nc.gpsimd)     # ot7
            _load_wqk_oc(0, 2, nc.scalar)     # ot2
            _load_wqk_oc(1, 2, nc.scalar)     # ot8
            for i in range(3):
                _q3[i].dma_start(out=wv[:, 2 * i:2 * i + 2, :],
                                 in_=wv_d[:, 2 * i:2 * i + 2, :])
            _load_wqk_oc(0, 3, nc.sync)       # ot3
            _load_wqk_oc(1, 3, nc.sync)       # ot9
            _load_wqk_oc(0, 4, nc.gpsimd)     # ot4
            _load_wqk_oc(1, 4, nc.gpsimd)     # ot10
            _load_wqk_oc(0, 5, nc.scalar)     # ot5
            _load_wqk_oc(1, 5, nc.scalar)     # ot11
            return xt, None

        def qkv_steps(b, xt, xt8):
            """Allocate qkt/v for batch b; return (qkt, v, steps) where each
            step emits one PE accumulation group + its ACT copy, so the qkv
            can be interleaved into the previous batch's attention."""
            qkt = qkt_pool.tile([128, 12, N], bf16, tag="qkt", name=f"qkt{b}")
            v = v_tiles[b % 2]
            steps = []

            def qk_fp8(ot, qc, qw):
                ps = ps_mm.tile([128, 512], f32, tag="mm", name="psq")
                for p in range(3):
                    nc.tensor.matmul(
                        ps[:, :qw],
                        lhsT=wqk8[p][:, :, 128 * ot:128 * (ot + 1)],
                        rhs=xt8[p][:, :, qc:qc + qw],
                        start=(p == 0), stop=(p == 2),
                        perf_mode=mybir.MatmulPerfMode.DoubleRow,
                    )
                nc.scalar.activation(
                    out=qkt[:, ot, qc:qc + qw], in_=ps[:, :qw],
                    func=mybir.ActivationFunctionType.Copy,
                )

            def qk_copy(ot, qc, ps):
                if have_qkb:
                    nc.scalar.activation(
                        out=qkt[:, ot, qc:qc + 288], in_=ps[:, :288],
                        func=mybir.ActivationFunctionType.Identity,
                        bias=qkb[:, ot:ot + 1],
                    )
                else:
                    nc.vector.tensor_copy(
                        out=qkt[:, ot, qc:qc + 288], in_=ps[:, :288],
                    )

            def qk_bf16(ot, qc):
                hf, oc = divmod(ot, 6)
                ps = ps_mm.tile([128, 512], f32, tag="mm", name="psq")
                for a in range(CA):
                    nc.tensor.matmul(
                        ps[:, :288],
                        lhsT=wqk[hf][:, oc, a, :],
                        rhs=xt[:, a, qc:qc + 288],
                        start=(a == 0), stop=(a == CA - 1),
                    )
                qk_copy(ot, qc, ps)

            def qk_bf16_ramp(ot):
                """Both qc chunks of one ot, a-major interleaved: during the
                startup DMA ramp, each newly arrived (xt[a], wqk[a]) tile
                feeds two matmuls instead of one, so the in-order PE is not
                blocked on the group's last-arriving tile."""
                hf, oc = divmod(ot, 6)
                ps0 = ps_mm.tile([128, 512], f32, tag="mm", name="psq")
                ps1 = ps_mm.tile([128, 512], f32, tag="mm", name="psq")
                for a in range(CA):
                    for qc, ps in ((0, ps0), (288, ps1)):
                        nc.tensor.matmul(
                            ps[:, :288],
                            lhsT=wqk[hf][:, oc, a, :],
                            rhs=xt[:, a, qc:qc + 288],
                            start=(a == 0), stop=(a == CA - 1),
                        )
                qk_copy(ot, 0, ps0)
                qk_copy(ot, 288, ps1)

            def v_step(kk, ch):
                tsz = TSZ[kk]
                ps = ps_mm.tile([128, 512], f32, tag="mm", name="psv")
                for a in range(CA):
                    nc.tensor.matmul(
                        ps[:tsz, :384],
                        lhsT=xt[:, a, 128 * kk:128 * kk + tsz],
                        rhs=wv[:, a, 384 * ch:384 * (ch + 1)],
                        start=(a == 0), stop=(a == CA - 1),
                    )
                nc.scalar.activation(
                    out=v[0:tsz, kk, 6 * ch:6 * (ch + 1), HD:2 * HD],
                    in_=ps[:tsz, :384],
                    func=mybir.ActivationFunctionType.Copy,
                )

            if use_fp8:
                for ot in range(12):
                    for (qc, qw) in ((0, 512), (512, 64)):
                        steps.append(lambda ot=ot, qc=qc, qw=qw: qk_fp8(ot, qc, qw))
            else:
                for ot in range(12):
                    for qc in (0, 288):
                        steps.append(lambda ot=ot, qc=qc: qk_bf16(ot, qc))
            for kk in range(5):
                for ch in range(2):
                    steps.append(lambda kk=kk, ch=ch: v_step(kk, ch))
            return qkt, v, steps, {"ramp": qk_bf16_ramp, "qk": qk_bf16,
                                   "v": v_step}

        def emit_pts_pair(qkt, j, fill=None):
            """S^T -> exp -> staircase mask for head pair (2j, 2j+1).

            The two heads' S^T matmuls are row-tiled onto PE row groups
            0:64 / 64:128 via tile_position so they run concurrently on
            hardware; their PSUM outputs share one 2-bank pair tile so a
            single Exp (and a single mask multiply) covers both heads.
            Returns 5 P^T pair tiles (plane i = head 2j+i), one per key
            tile kk, each covering q columns [128*kk, N) at local offset.
            """
            ptps = []
            for kk in range(5):
                ksz = KSZ[kk]
                ko = 128 * kk
                ptp = pt_pool.tile([128, 2, N - ko], bf16, tag=f"pt{kk}",
                                   name=f"pt{kk}")
                chunks = S_CHUNKS[kk]
                pss = [ps_s.tile([128, 2, 512], f32, tag="s", name=f"s{ci}")
                       for ci in range(len(chunks))]
                for i in range(2):
                    po = 64 * i
                    for ps, (qo, qw) in zip(pss, chunks):
                        nc.tensor.matmul(
                            ps[0:ksz, i, 0:qw],
                            lhsT=qkt[po:po + 64, 6 + j, ko:ko + ksz],
                            rhs=qkt[po:po + 64, j, qo:qo + qw],
                            start=True, stop=True,
                            tile_position=(po, 0),
                        )
                for ps, (qo, qw) in zip(pss, chunks):
                    nc.scalar.activation(
                        out=ptp[0:ksz, :, qo - ko:qo - ko + qw],
                        in_=ps[0:ksz, :, 0:qw],
                        func=mybir.ActivationFunctionType.Exp,
                        scale=float(exp_scale),
                    )
                # staircase mask on the diagonal tile. Pool only: DVE runs
                # the norm-TT bursts, which would delay the mask and stall
                # the next pair's AV matmuls behind it.
                eng = nc.gpsimd
                eng.tensor_tensor(
                    out=ptp[0:ksz, :, 0:ksz],
                    in0=ptp[0:ksz, :, 0:ksz],
                    in1=maskT2[0:ksz, :, 0:ksz],
                    op=mult,
                )
                ptps.append(ptp)
                if fill is not None:
                    fill(1)
            return ptps

        def emit_av(v, h, ptps, at, j2):
            """AV matmuls + inline softmax normalization straight from PSUM.

            PSUM rows 0:64 hold the denominator (replicated via V's ones
            columns 0:HD), rows 64:128 hold A^T_unnorm. ps0 covers q in
            [64, 576) so all five key tiles fold into one accumulation
            group (key tile kk contributes q >= 128*kk); the remaining q in
            [0, 64) needs only key tile 0 (keys 64:127 are mask-zeroed for
            those queries), a single extra matmul. The DVE reciprocal reads
            the denominator rows at PSUM base 0 (custom-DVE ops require
            base-partition-0 operands); the normalize tensor_tensor reads
            A^T_unnorm as a PSUM operand (base-free) so no staging copy is
            needed - its output lands directly in the bf16 A^T tile."""
            hh = h % 2
            ps0 = ps_av.tile([128, 512], f32, tag="av")
            for kk in range(5):
                ksz = KSZ[kk]
                ko = 128 * kk
                qo = max(64, ko)
                nc.tensor.matmul(
                    ps0[:, qo - 64:512],
                    lhsT=v[0:ksz, kk, h, :],
                    rhs=ptps[kk][0:ksz, hh, qo - ko:576 - ko],
                    start=(kk == 0), stop=(kk == 4),
                )
            ps1 = ps_av.tile([128, 512], f32, tag="av", name="ps1")
            nc.tensor.matmul(
                ps1[:, 0:64],
                lhsT=v[0:128, 0, h, :],
                rhs=ptps[0][0:128, hh, 0:64],
                start=True, stop=True,
            )
            drecp = d_pool.tile([64, N], f32, tag="drecp")
            nc.vector.reciprocal_approx_fast(out=drecp[:, 64:576],
                                             in_=ps0[0:64, 0:512])
            nc.vector.tensor_tensor(
                out=at[64 * hh:64 * hh + 64, j2, 64:576],
                in0=ps0[64:128, 0:512],
                in1=drecp[:, 64:576],
                op=mult,
            )
            nc.vector.reciprocal_approx_fast(out=drecp[:, 0:64],
                                             in_=ps1[0:64, 0:64])
            nc.vector.tensor_tensor(
                out=at[64 * hh:64 * hh + 64, j2, 0:64],
                in0=ps1[64:128, 0:64],
                in1=drecp[:, 0:64],
                op=mult,
            )

        def emit_attention(b, qkt, v, at, filler=None, carry=0, eager=0,
                           hold=0, av_lag=1):
            """All heads, software-pipelined at head-pair granularity:
            S/exp/mask of pair j+1 is emitted before AV of pair j so the
            PE's in-order stream never waits on the ACT chain; pair
            normalization follows the pair's second AV. `filler` steps
            (tagged ("q", fn) for next-batch qkv — must finish here — or
            ("p", fn) for prev-batch proj) are spread between pairs to keep
            the PE dense; up to `carry` proj steps are left unconsumed and
            returned so the next (more starved) attention can use them."""
            filler = list(filler or [])
            supply = len(filler) - carry
            # fill slots: 5 S chunks/pair + 3 per AV pair (one between the
            # two heads' AVs so the PE has work while the DVE drains the
            # first head's PSUM banks) + 3 at the tail.
            total_slots = 5 * CA + 3 * CA + 3
            state = [0, 0]          # slots seen, steps consumed

            def fill(n, tail=False):
                state[0] += n
                target = min(supply - hold, min(eager, state[0])
                             + ((supply - eager) * state[0]) // total_slots)
                while state[1] < target and filler:
                    filler.pop(0)[1]()
                    state[1] += 1

            def fill_tail(n):
                fill(n)

            def flush_end():
                # all qkv steps must be emitted inside this attention (the
                # next batch's S matmuls precede them in PE program order);
                # leave up to `carry` proj steps for the next attention.
                keep = []
                n_proj = sum(1 for t, _ in filler if t == "p")
                for t, fn in filler:
                    if t == "q":
                        fn()
                    elif n_proj > carry:
                        fn()
                        n_proj -= 1
                    else:
                        keep.append((t, fn))
                return keep

            ptps = {}

            def do_av(j):
                ptp = ptps.pop(j)
                emit_av(v, 2 * j, ptp, at, j)
                fill(1)
                emit_av(v, 2 * j + 1, ptp, at, j)
                fill(2)

            for j in range(CA):
                ptps[j] = emit_pts_pair(qkt, j, fill=fill)
                if debug_taps and b == 0:
                    for kk in range(2):
                        nc.sync.dma_start(out=pt_tap[j, kk][:, 0:N - 128 * kk],
                                          in_=ptps[j][kk])
                if j >= av_lag:
                    do_av(j - av_lag)
            for j in range(CA - av_lag, CA):
                do_av(j)
            fill_tail(3)
            return flush_end()

        def proj_steps(b, at):
            """Proj work as a list of closures so it can be interleaved into
            the next batch's attention emission as PE filler."""
            steps = []
            for tt in range(5):
                for ch in range(2):
                    steps.append(("mm", b, at, tt, ch))
                    steps.append(("dma", b, tt, ch))
            return steps

        _ysb = {}

        def run_proj_step(step):
            kind = step[0]
            if kind == "mm":
                _, b, at, tt, ch = step
                tsz = TSZ[tt]
                to = 128 * tt
                if (b, tt) not in _ysb:
                    _ysb[(b, tt)] = y_pool.tile([128, C], bf16, tag="y",
                                                name=f"ysb{b}_{tt}")
                ysb = _ysb[(b, tt)]
                ps = ps_mm.tile([128, 512], f32, tag="mm", name="psp")
                for a in range(CA):
                    nc.tensor.matmul(
                        ps[:tsz, :384],
                        lhsT=at[:, a, to:to + tsz],
                        rhs=wp[:, a, 384 * ch:384 * (ch + 1)],
                        start=(a == 0), stop=(a == CA - 1),
                    )
                nc.vector.tensor_copy(
                    out=ysb[0:tsz, 384 * ch:384 * (ch + 1)],
                    in_=ps[:tsz, :384],
                )
            else:
                _, b, tt, ch = step
                tsz = TSZ[tt]
                to = 128 * tt
                ysb = _ysb[(b, tt)]
                if ch == 1:
                    _ysb.pop((b, tt))
                eng = nc.sync
                if b == BL - 1:
                    eng = (nc.sync, nc.gpsimd, nc.scalar)[(2 * tt + ch) % 3]
                eng.dma_start(
                    out=y_d[N * b + to:N * b + to + tsz,
                            384 * ch:384 * (ch + 1)],
                    in_=ysb[0:tsz, 384 * ch:384 * (ch + 1)],
                )

        def emit_proj(b, at):
            for step in proj_steps(b, at):
                run_proj_step(step)

        def interleave(a_list, b_list, ratio=2):
            """ratio a-items per b-item until either runs dry."""
            out = []
            ai = bi = 0
            while ai < len(a_list) or bi < len(b_list):
                for _ in range(ratio):
                    if ai < len(a_list):
                        out.append(a_list[ai])
                        ai += 1
                if bi < len(b_list):
                    out.append(b_list[bi])
                    bi += 1
            return out

        # software-pipelined batch loop: batch b+1's qkv and batch b-1's
        # proj are chopped into steps and threaded into batch b's attention
        # at every stall point, so the PE's in-order stream stays dense.
        xt, xt8 = emit_startup_loads()
        emit_weight_loads()
        # warm-up matmuls on a zeroed tile: fill the initial DMA-latency gap
        # with PE activity so the HAM clock gate reaches full speed before
        # the first real accumulation group, at no dependency cost.
        for _ in range(8):
            psw = ps_mm.tile([128, 512], f32, tag="mm", name="psw")
            nc.tensor.matmul(psw[:, :512], lhsT=warm[:, 0:128],
                             rhs=warm[:, 0:512], start=True, stop=True)
        qkt, v, _, fns = qkv_steps(0, xt, xt8)
        # prologue: Q/K projections for head pair 0 in ramp (a-major) order
        # so matmuls start as each x/wqk contraction tile lands; the rest of
        # batch 0's qkv is consumed eagerly (1 step/slot) inside attention 0.
        fns["ramp"](0)
        fns["ramp"](6)
        b0_rest = []

        def _q0(ot):
            for qc in (0, 288):
                b0_rest.append(("q", lambda ot=ot, qc=qc: fns["qk"](ot, qc)))

        def _v0(ch):
            for kk in range(5):
                b0_rest.append(("q", lambda kk=kk, ch=ch: fns["v"](kk, ch)))

        def _w0(n):
            def step():
                for _ in range(n):
                    psw = ps_mm.tile([128, 512], f32, tag="mm", name="psw")
                    nc.tensor.matmul(psw[:, :512], lhsT=warm[:, 0:128],
                                     rhs=warm[:, 0:512], start=True, stop=True)
            b0_rest.append(("q", step))

        _q0(1); _q0(7); _q0(2); _q0(8); _q0(3); _q0(9); _v0(0)
        _q0(4); _q0(10); _v0(1); _q0(5); _q0(11)
        pending = None
        carry_steps = []
        # proj steps carried from attention b to b+1: the last batch has no
        # next-batch qkv filler, so hold back some of the earlier proj work
        CARRY = {BL - 3: 12, BL - 2: 26}
        for b in range(BL):
            if debug_taps and b == 0:
                nc.sync.dma_start(out=qkt_tap[:, :, :], in_=qkt)
            filler = list(carry_steps)
            if pending is not None:
                filler += [("p", lambda s=s: run_proj_step(s))
                           for s in proj_steps(pending[0], pending[1])]
            nxt = None
            if b + 1 < BL:
                xt, xt8 = emit_xt(b + 1)
                qkt1, v1, qsteps, _ = qkv_steps(b + 1, xt, xt8)
                nxt = (qkt1, v1)
                filler = interleave([("q", s) for s in qsteps], filler,
                                    ratio=2)
            eager = 0
            if b == 0:
                filler = b0_rest + filler
                eager = len(b0_rest)
            at = at_pool.tile([128, CA, N], bf16, tag="at")
            carry_steps = emit_attention(b, qkt, v, at, filler=filler,
                                         carry=CARRY.get(b, 0), eager=eager,
                                         hold=6 if b == BL - 1 else 0,
                                         av_lag=2 if b == 0 else 1)
            pending = (b, at)
            if nxt is not None:
                qkt, v = nxt
        for _, fn in carry_steps:
            fn()
        emit_proj(pending[0], pending[1])

    nc.compile()
    return nc


def _get_program(have_qkb=False):
    key = ("nc", have_qkb)
    if key not in _prog_cache:
        _prog_cache[key] = _build_program(have_qkb)
    return _prog_cache[key]


def kernel(x, qkv_w, qkv_b, proj_w, proj_b):
    global LAST_EXEC_NS, LAST_TRACE_PATH
    from concourse.bass_utils import run_bass_kernel_spmd

    x = np.asarray(x, np.float32)
    qkv_w = np.asarray(qkv_w, np.float32)
    qkv_b = np.asarray(qkv_b, np.float32)
    proj_w = np.asarray(proj_w, np.float32)
    proj_b = np.asarray(proj_b, np.float32)

    perm = _block_perm()
    x_s = x[:, perm, :]

    have_qkb = bool(np.any(qkv_b[:2 * C]))
    use_fp8 = USE_FP8 and not have_qkb
    wqkT32 = np.ascontiguousarray(qkv_w[:2 * C].T)          # [C, 2C] f32
    wvT = np.ascontiguousarray(
        qkv_w[2 * C:].T.astype(BF16).reshape(CA, 128, C).transpose(1, 0, 2))
    wpT = np.ascontiguousarray(
        proj_w.T.astype(BF16).reshape(CA, 128, C).transpose(1, 0, 2))
    idx = np.arange(128)
    mask1 = (idx[:, None] // 16 <= idx[None, :] // 16).astype(BF16)
    maskT = np.ascontiguousarray(
        np.broadcast_to(mask1[:, None, :], (128, 2, 128)))

    shared = {"wvT": wvT, "wpT": wpT, "maskT": maskT}
    if use_fp8:
        # wqk8[p, r, i, o] = WS * wqkT[256p + 128i + r, o] in fp8 e4m3
        w8 = np.clip(wqkT32 * WS, -240, 240).astype(FP8)
        shared["wqk8"] = np.ascontiguousarray(
            w8.reshape(3, 2, 128, 2 * C).transpose(0, 2, 1, 3))
    else:
        shared["wqkT"] = np.ascontiguousarray(
            wqkT32.astype(BF16).reshape(CA, 128, 2, 6, 128)
            .transpose(2, 3, 1, 0, 4))
    if have_qkb:
        shared["qkb"] = np.ascontiguousarray(
            qkv_b[:2 * C].reshape(12, 128).T).astype(np.float32)
    in_maps = []
    for c in range(CORES):
        xb = x_s[BL * c:BL * (c + 1)]                        # [BL, N, C]
        xbT = np.ascontiguousarray(xb.transpose(0, 2, 1))    # [BL, C, N] f32
        m = {"xT": np.ascontiguousarray(
            xbT.astype(BF16).reshape(BL, CA, 128, N).transpose(0, 2, 1, 3))}
        m.update(shared)
        in_maps.append(m)

    nc = _get_program(have_qkb)
    res = None
    last_err = None
    for attempt in range(4):
        try:
            res = run_bass_kernel_spmd(nc, in_maps, core_ids=list(range(CORES)),
                                       trace=TRACE and attempt == 0)
            break
        except Exception as e:  # transient NRT/axon failures - retry
            last_err = e
            import time
            time.sleep(5 * (attempt + 1))
    if res is None:
        raise last_err
    LAST_EXEC_NS = res.exec_time_ns
    LAST_TRACE_PATH = (res.instructions_and_trace[1]
                       if res.instructions_and_trace else None)

    y_s = np.empty((B, N, C), np.float32)
    for c in range(CORES):
        y_s[BL * c:BL * (c + 1)] = res.results[c]["y"].astype(
            np.float32).reshape(BL, N, C)
    y = np.empty_like(y_s)
    y[:, perm, :] = y_s
    # v-bias and proj-bias contribute a constant per-channel vector to every
    # token: fold them in exactly here (attention rows sum to 1).
    tail = proj_b.astype(np.float64) + qkv_b[2 * C:].astype(np.float64) @ proj_w.T.astype(np.float64)
    if np.any(tail):
        y += tail.astype(np.float32)[None, None, :]
    return y



# revision 57
# speedup vs baseline: 1.0064x; 1.0064x over previous
"""Block-sparse attention (ViT-style block-causal) on 8 Trainium2 NeuronCores.

Strategy: data-parallel over batch (4 batches per core), SPMD, no collectives.

Math (per batch, tokens pre-permuted on host into block-sorted order so the
mask is block-causal with 16-token blocks):
  - qkv projection computed directly in transposed layouts:
      QT/KT [o, t] = wqkT.T @ xT   (o on partitions, 2 heads per 128-tile)
      V     [t, o] = xT.T  @ wvT   (natural layout, written into columns
      HD:2*HD of a persistent tile whose columns 0:HD are constant 1.0)
  - S^T[k, q] = KT.T @ QT per head (contraction over head dim = 64)
  - P^T = exp(scale * S^T)  (no max-subtraction; scores bounded ~|6.5|)
    multiplied by a 16-granular staircase 0/1 mask on the diagonal 128-tile
  - AV matmul with lhsT = [ones | V] (full 128 PE columns): PSUM rows
    0:64 come out as the softmax denominator sum_k P^T replicated across
    64 partitions, rows 64:128 as A^T_unnorm. The DVE reciprocal reads
    the denominator rows straight from PSUM at base partition 0 (custom-
    DVE ops require base-0 operands) and the normalize tensor_tensor
    reads A^T_unnorm as a (base-free) PSUM operand, writing bf16 A^T
    directly - no staging copies, no cross-partition broadcast, no
    selector matmul, no denominator DMA.
  - y[t, o] = A^T.T @ wpT  (A^T layout feeds proj directly, transpose-
    free); y stored bf16 (halves the store traffic; host upcasts).

All matmul operands bf16 (fp32 PSUM accumulation).

Key optimizations over the straightforward schedule:
  - The two heads of a pair compute S^T concurrently on hardware: their
    K=64 matmuls are row-tiled onto PE row groups 0:64 / 64:128 via
    tile_position (the 128x128 array is 16 independent 32x32 subarrays),
    writing adjacent PSUM banks of one [128, 2, 512] pair tile.
  - One Exp and one staircase-mask multiply cover both heads of a pair
    through 3D [*, 2, *] access patterns, halving the ACT/Pool per-
    instruction access overhead (185ns/instr on ACT).
  - Deep software pipelining: batch b+1's qkv and batch b-1's proj are
    chopped into single-accumulation-group steps and threaded into batch
    b's attention at every stall point (after each S chunk / between and
    after the two AVs of a pair), paced uniformly against the available
    slots; unconsumed proj steps carry forward to the (filler-starved)
    later batches, with a few held all the way to the final attention's
    flush to cover the last pair-norm drain before the final proj.
  - Startup: framework preamble ends ~7us and DMA bandwidth is only
    ~200GB/s aggregate, so the ramp is choreographed byte-by-byte: the
    wqk weights are stored oc-tile-major in DRAM and loaded in exactly
    the order the qk groups consume them, spread over the sync/gpsimd/
    scalar queues; batch 0's first Q/K groups run a-major (ramp) so
    matmuls start as each contraction tile lands; the rest of batch 0's
    qkv is consumed eagerly (1 step/slot) inside attention 0, whose
    first AV is deferred one extra pair (av_lag=2) to buy time for the
    wv load; 8 warm-up matmuls on a zeroed tile (memset emitted as the
    first DVE instruction so nothing queues ahead of it) bring the PE
    HAM clock gate toward 2.4 GHz before real work.
  - Engine assignment: PE matmuls; ACT exp + v copies; DVE qk copies +
    reciprocal + normalize + proj copies; Pool (GpSimd) all staircase
    masks; y DMAs column-split on the sync queue, spread over all three
    DMA queues for the last batch (mask/exp engines are idle by then).
"""

import numpy as np
import ml_dtypes

B, N, C, H = 32, 576, 768, 12
HD = C // H                      # 64
CORES = 8
BL = B // CORES                  # 4 batches per core
T = BL * N                       # 2304 tokens per core
GRID, BS = 24, 4
SCALE = HD ** -0.5
CA = C // 128                    # 6 contraction tiles
BF16 = ml_dtypes.bfloat16

# q-chunks of the S^T matmul for each key tile kk: (q_offset, width)
S_CHUNKS = {
    0: [(0, 512), (512, 64)],
    1: [(128, 448)],
    2: [(256, 320)],
    3: [(384, 192)],
    4: [(512, 64)],
}
KSZ = [128, 128, 128, 128, 64]   # key-tile sizes (576 = 4*128 + 64)
TSZ = KSZ                        # token-tile sizes within a batch

FP8 = ml_dtypes.float8_e4m3        # TRN FP8_EXP4: max normal +-240
XS = 16.0                          # x pre-scale before fp8 cast
WS = 256.0                         # wqk pre-scale before fp8 cast
FP8_QS = XS * WS                   # Q/K carry this extra factor in fp8 mode
USE_FP8 = False                    # fp8 QK noise ~5% L2 - over the 2e-2 gate

TRACE = False
LAST_EXEC_NS = None
LAST_TRACE_PATH = None

_prog_cache = {}


def _block_perm():
    r = np.arange(GRID) // BS
    bi = (r[:, None] * (GRID // BS) + r[None, :]).reshape(-1)
    return np.argsort(bi, kind="stable")


def _build_program(have_qkb=False, debug_taps=False):
    from contextlib import ExitStack

    import concourse.mybir as mybir
    import concourse.tile as tile
    from concourse import bacc

    dt = mybir.dt
    f32 = dt.float32
    bf16 = dt.bfloat16
    fp8 = dt.float8e4
    mult = mybir.AluOpType.mult
    use_fp8 = USE_FP8 and not have_qkb
    exp_scale = SCALE / (FP8_QS * FP8_QS) if use_fp8 else SCALE

    nc = bacc.Bacc("TRN2", target_bir_lowering=False, debug=False,
                   num_devices=CORES)

    xT_d = nc.dram_tensor("xT", [BL, 128, CA, N], bf16, kind="ExternalInput").ap()
    if use_fp8:
        # contraction dim pair-interleaved for DoubleRow: [pair, 128, plane, .]
        xT8_d = nc.dram_tensor("xT8", [BL, 3, 128, 2, N], fp8,
                               kind="ExternalInput").ap()
        wqk8_d = nc.dram_tensor("wqk8", [3, 128, 2, 2 * C], fp8,
                                kind="ExternalInput").ap()
    else:
        wqk_d = nc.dram_tensor("wqkT", [2, 6, 128, CA, 128], bf16,
                               kind="ExternalInput").ap()
    wv_d = nc.dram_tensor("wvT", [128, CA, C], bf16, kind="ExternalInput").ap()
    wp_d = nc.dram_tensor("wpT", [128, CA, C], bf16, kind="ExternalInput").ap()
    mask_d = nc.dram_tensor("maskT", [128, 2, 128], bf16, kind="ExternalInput").ap()
    y_d = nc.dram_tensor("y", [T, C], bf16, kind="ExternalOutput").ap()
    if have_qkb:
        qkb_d = nc.dram_tensor("qkb", [128, 12], f32, kind="ExternalInput").ap()
    if debug_taps:
        qkt_tap = nc.dram_tensor("qkt_tap", [128, 12, N], bf16,
                                 kind="ExternalOutput").ap()
        pt_tap = nc.dram_tensor("pt_tap", [CA, 2, 128, 2, N], bf16,
                                kind="ExternalOutput").ap()
        atu_tap = nc.dram_tensor("atu_tap", [H, HD + 1, N], f32,
                                 kind="ExternalOutput").ap()

    with tile.TileContext(nc) as tc, ExitStack() as ctx:
        consts = ctx.enter_context(tc.tile_pool(name="consts", bufs=1))
        xt_pool = ctx.enter_context(tc.tile_pool(name="xt", bufs=2))
        qkt_pool = ctx.enter_context(tc.tile_pool(name="qkt", bufs=2))
        pt_pool = ctx.enter_context(tc.tile_pool(name="pt", bufs=5))
        at_pool = ctx.enter_context(tc.tile_pool(name="at", bufs=2))
        d_pool = ctx.enter_context(tc.tile_pool(name="d", bufs=4))
        y_pool = ctx.enter_context(tc.tile_pool(name="y", bufs=6))

        ps_mm = ctx.enter_context(tc.tile_pool(name="ps_mm", bufs=2, space="PSUM"))
        ps_s = ctx.enter_context(tc.tile_pool(name="ps_s", bufs=2, space="PSUM"))
        ps_av = ctx.enter_context(tc.tile_pool(name="ps_av", bufs=2, space="PSUM"))

        # ---- constants / weights (per-contraction-tile tiles so the first
        # matmuls only wait on the first DMA) ----
        if use_fp8:
            wqk8 = [consts.tile([128, 2, 2 * C], fp8, tag=f"wqk8_{p}",
                                name=f"wqk8_{p}") for p in range(3)]
        else:
            # split by output-channel halves so the first 6 ot groups only
            # wait on half the weight bytes; contraction tiles joined so
            # loads go out in 2-tile DMAs (halves HWDGE issue serialization)
            wqk = [consts.tile([128, 6, CA, 128], bf16, tag=f"wqkh{hf}",
                               name=f"wqkh{hf}") for hf in range(2)]
        wv = consts.tile([128, CA, C], bf16, tag="wv", name="wv")
        wp = consts.tile([128, CA, C], bf16, tag="wp", name="wp")
        maskT2 = consts.tile([128, 2, 128], bf16, tag="mask")
        # persistent double-buffered V (by batch parity): [token, kk, head,
        # 2*HD] where columns 0:HD are constant 1.0 and HD:2*HD hold V, so
        # every AV matmul (lhsT [ksz, 128], full PE width) replicates the
        # softmax denominator sum_k P^T across output partitions 0:64 (base
        # 0, as the DVE custom reciprocal requires) with A^T_unnorm on
        # partitions 64:128. Reciprocal and normalize both read the PSUM
        # directly - no A^T staging copies, no cross-partition broadcast,
        # no E-matmul, no denominator DMA.
        warm = consts.tile([128, 512], bf16, tag="warm")
        nc.vector.memset(warm, 0.0)
        v_tiles = [consts.tile([128, 5, H, 2 * HD], bf16, tag=f"v{i}",
                               name=f"v{i}") for i in range(2)]
        for vt in v_tiles:
            for kk in range(5):
                nc.vector.memset(vt[:, kk, :, 0:HD], 1.0)

        _dma_engines = [nc.sync]

        _q3 = [nc.sync, nc.gpsimd, nc.scalar]

        def emit_weight_loads():
            # wp rides each queue behind the critical startup set
            for i in range(3):
                _q3[i].dma_start(
                    out=wp[:, 2 * i:2 * i + 2, :],
                    in_=wp_d[:, 2 * i:2 * i + 2, :])
        if have_qkb:
            qkb = consts.tile([128, 12], f32, tag="qkb")
            nc.sync.dma_start(out=qkb, in_=qkb_d[:, :])

        def emit_xt(b, eng=None):
            xt = xt_pool.tile([128, CA, N], bf16, tag="xt", name="xt")
            for i in range(3):
                e = eng or (nc.sync if i % 2 == 0 else nc.gpsimd)
                e.dma_start(
                    out=xt[:, 2 * i:2 * i + 2, :],
                    in_=xT_d[b, :, 2 * i:2 * i + 2, :])
            return xt, None

        def _load_wqk_oc(hf, oc, eng):
            # one output-channel tile (the weights of qk group ot=6*hf+oc),
            # all contraction tiles: 197KB, so weight chunks land in the
            # order the qk groups consume them (startup is DMA-BW-bound at
            # ~200GB/s aggregate; per-queue ~70GB/s when all three run).
            eng.dma_start(out=wqk[hf][:, oc], in_=wqk_d[hf, oc])

        def emit_startup_loads():
            """Batch 0's x tiles + the wqk oc-tiles in consumption order,
            spread across three DMA queues (sync/gpsimd/scalar)."""
            xt = xt_pool.tile([128, CA, N], bf16, tag="xt", name="xt")
            # queue i: x chunk first (ramp rhs), then its oc-tiles
            _load_wqk_oc(0, 0, nc.sync)
            nc.scalar.dma_start(out=maskT2, in_=mask_d[:, :, :])
            for i in range(3):
                _q3[i].dma_start(out=xt[:, 2 * i:2 * i + 2, :],
                                 in_=xT_d[0, :, 2 * i:2 * i + 2, :])
            _load_wqk_oc(1, 0, nc.sync)       # ot6 (ramp pair 0 K)
            _load_wqk_oc(0, 1, nc.gpsimd)     # ot1
            _load_wqk_oc(1, 1, nc.gpsimd)     # ot7
            _load_wqk_oc(0, 2, nc.scalar)     # ot2
            _load_wqk_oc(1, 2, nc.scalar)     # ot8
            for i in range(3):
                _q3[i].dma_start(out=wv[:, 2 * i:2 * i + 2, :],
                                 in_=wv_d[:, 2 * i:2 * i + 2, :])
            _load_wqk_oc(0, 3, nc.sync)       # ot3
            _load_wqk_oc(1, 3, nc.sync)       # ot9
            _load_wqk_oc(0, 4, nc.gpsimd)     # ot4
            _load_wqk_oc(1, 4, nc.gpsimd)     # ot10
            _load_wqk_oc(0, 5, nc.scalar)     # ot5
            _load_wqk_oc(1, 5, nc.scalar)     # ot11
            return xt, None

        def qkv_steps(b, xt, xt8):
            """Allocate qkt/v for batch b; return (qkt, v, steps) where each
            step emits one PE accumulation group + its ACT copy, so the qkv
            can be interleaved into the previous batch's attention."""
            qkt = qkt_pool.tile([128, 12, N], bf16, tag="qkt", name=f"qkt{b}")
            v = v_tiles[b % 2]
            steps = []

            def qk_fp8(ot, qc, qw):
                ps = ps_mm.tile([128, 512], f32, tag="mm", name="psq")
                for p in range(3):
                    nc.tensor.matmul(
                        ps[:, :qw],
                        lhsT=wqk8[p][:, :, 128 * ot:128 * (ot + 1)],
                        rhs=xt8[p][:, :, qc:qc + qw],
                        start=(p == 0), stop=(p == 2),
                        perf_mode=mybir.MatmulPerfMode.DoubleRow,
                    )
                nc.scalar.activation(
                    out=qkt[:, ot, qc:qc + qw], in_=ps[:, :qw],
                    func=mybir.ActivationFunctionType.Copy,
                )

            def qk_copy(ot, qc, ps):
                if have_qkb:
                    nc.scalar.activation(
                        out=qkt[:, ot, qc:qc + 288], in_=ps[:, :288],
                        func=mybir.ActivationFunctionType.Identity,
                        bias=qkb[:, ot:ot + 1],
                    )
                else:
                    nc.vector.tensor_copy(
                        out=qkt[:, ot, qc:qc + 288], in_=ps[:, :288],
                    )

            def qk_bf16(ot, qc):
                hf, oc = divmod(ot, 6)
                ps = ps_mm.tile([128, 512], f32, tag="mm", name="psq")
                for a in range(CA):
                    nc.tensor.matmul(
                        ps[:, :288],
                        lhsT=wqk[hf][:, oc, a, :],
                        rhs=xt[:, a, qc:qc + 288],
                        start=(a == 0), stop=(a == CA - 1),
                    )
                qk_copy(ot, qc, ps)

            def qk_bf16_ramp(ot):
                """Both qc chunks of one ot, a-major interleaved: during the
                startup DMA ramp, each newly arrived (xt[a], wqk[a]) tile
                feeds two matmuls instead of one, so the in-order PE is not
                blocked on the group's last-arriving tile."""
                hf, oc = divmod(ot, 6)
                ps0 = ps_mm.tile([128, 512], f32, tag="mm", name="psq")
                ps1 = ps_mm.tile([128, 512], f32, tag="mm", name="psq")
                for a in range(CA):
                    for qc, ps in ((0, ps0), (288, ps1)):
                        nc.tensor.matmul(
                            ps[:, :288],
                            lhsT=wqk[hf][:, oc, a, :],
                            rhs=xt[:, a, qc:qc + 288],
                            start=(a == 0), stop=(a == CA - 1),
                        )
                qk_copy(ot, 0, ps0)
                qk_copy(ot, 288, ps1)

            def v_step(kk, ch):
                tsz = TSZ[kk]
                ps = ps_mm.tile([128, 512], f32, tag="mm", name="psv")
                for a in range(CA):
                    nc.tensor.matmul(
                        ps[:tsz, :384],
                        lhsT=xt[:, a, 128 * kk:128 * kk + tsz],
                        rhs=wv[:, a, 384 * ch:384 * (ch + 1)],
                        start=(a == 0), stop=(a == CA - 1),
                    )
                nc.scalar.activation(
                    out=v[0:tsz, kk, 6 * ch:6 * (ch + 1), HD:2 * HD],
                    in_=ps[:tsz, :384],
                    func=mybir.ActivationFunctionType.Copy,
                )

            if use_fp8:
                for ot in range(12):
                    for (qc, qw) in ((0, 512), (512, 64)):
                        steps.append(lambda ot=ot, qc=qc, qw=qw: qk_fp8(ot, qc, qw))
            else:
                for ot in range(12):
                    for qc in (0, 288):
                        steps.append(lambda ot=ot, qc=qc: qk_bf16(ot, qc))
            for kk in range(5):
                for ch in range(2):
                    steps.append(lambda kk=kk, ch=ch: v_step(kk, ch))
            return qkt, v, steps, {"ramp": qk_bf16_ramp, "qk": qk_bf16,
                                   "v": v_step}

        def emit_pts_pair(qkt, j, fill=None):
            """S^T -> exp -> staircase mask for head pair (2j, 2j+1).

            The two heads' S^T matmuls are row-tiled onto PE row groups
            0:64 / 64:128 via tile_position so they run concurrently on
            hardware; their PSUM outputs share one 2-bank pair tile so a
            single Exp (and a single mask multiply) covers both heads.
            Returns 5 P^T pair tiles (plane i = head 2j+i), one per key
            tile kk, each covering q columns [128*kk, N) at local offset.
            """
            ptps = []
            for kk in range(5):
                ksz = KSZ[kk]
                ko = 128 * kk
                ptp = pt_pool.tile([128, 2, N - ko], bf16, tag=f"pt{kk}",
                                   name=f"pt{kk}")
                chunks = S_CHUNKS[kk]
                pss = [ps_s.tile([128, 2, 512], f32, tag="s", name=f"s{ci}")
                       for ci in range(len(chunks))]
                for i in range(2):
                    po = 64 * i
                    for ps, (qo, qw) in zip(pss, chunks):
                        nc.tensor.matmul(
                            ps[0:ksz, i, 0:qw],
                            lhsT=qkt[po:po + 64, 6 + j, ko:ko + ksz],
                            rhs=qkt[po:po + 64, j, qo:qo + qw],
                            start=True, stop=True,
                            tile_position=(po, 0),
                        )
                for ps, (qo, qw) in zip(pss, chunks):
                    nc.scalar.activation(
                        out=ptp[0:ksz, :, qo - ko:qo - ko + qw],
                        in_=ps[0:ksz, :, 0:qw],
                        func=mybir.ActivationFunctionType.Exp,
                        scale=float(exp_scale),
                    )
                # staircase mask on the diagonal tile. Pool only: DVE runs
                # the norm-TT bursts, which would delay the mask and stall
                # the next pair's AV matmuls behind it.
                eng = nc.gpsimd
                eng.tensor_tensor(
                    out=ptp[0:ksz, :, 0:ksz],
                    in0=ptp[0:ksz, :, 0:ksz],
                    in1=maskT2[0:ksz, :, 0:ksz],
                    op=mult,
                )
                ptps.append(ptp)
                if fill is not None:
                    fill(1)
            return ptps

        def emit_av(v, h, ptps, at, j2):
            """AV matmuls + inline softmax normalization straight from PSUM.

            PSUM rows 0:64 hold the denominator (replicated via V's ones
            columns 0:HD), rows 64:128 hold A^T_unnorm. ps0 covers q in
            [64, 576) so all five key tiles fold into one accumulation
            group (key tile kk contributes q >= 128*kk); the remaining q in
            [0, 64) needs only key tile 0 (keys 64:127 are mask-zeroed for
            those queries), a single extra matmul. The DVE reciprocal reads
            the denominator rows at PSUM base 0 (custom-DVE ops require
            base-partition-0 operands); the normalize tensor_tensor reads
            A^T_unnorm as a PSUM operand (base-free) so no staging copy is
            needed - its output lands directly in the bf16 A^T tile."""
            hh = h % 2
            ps0 = ps_av.tile([128, 512], f32, tag="av")
            for kk in range(5):
                ksz = KSZ[kk]
                ko = 128 * kk
                qo = max(64, ko)
                nc.tensor.matmul(
                    ps0[:, qo - 64:512],
                    lhsT=v[0:ksz, kk, h, :],
                    rhs=ptps[kk][0:ksz, hh, qo - ko:576 - ko],
                    start=(kk == 0), stop=(kk == 4),
                )
            ps1 = ps_av.tile([128, 512], f32, tag="av", name="ps1")
            nc.tensor.matmul(
                ps1[:, 0:64],
                lhsT=v[0:128, 0, h, :],
                rhs=ptps[0][0:128, hh, 0:64],
                start=True, stop=True,
            )
            drecp = d_pool.tile([64, N], f32, tag="drecp")
            nc.vector.reciprocal_approx_fast(out=drecp[:, 64:576],
                                             in_=ps0[0:64, 0:512])
            nc.vector.tensor_tensor(
                out=at[64 * hh:64 * hh + 64, j2, 64:576],
                in0=ps0[64:128, 0:512],
                in1=drecp[:, 64:576],
                op=mult,
            )
            nc.vector.reciprocal_approx_fast(out=drecp[:, 0:64],
                                             in_=ps1[0:64, 0:64])
            nc.vector.tensor_tensor(
                out=at[64 * hh:64 * hh + 64, j2, 0:64],
                in0=ps1[64:128, 0:64],
                in1=drecp[:, 0:64],
                op=mult,
            )

        def emit_attention(b, qkt, v, at, filler=None, carry=0, eager=0,
                           hold=0, av_lag=1):
            """All heads, software-pipelined at head-pair granularity:
            S/exp/mask of pair j+1 is emitted before AV of pair j so the
            PE's in-order stream never waits on the ACT chain; pair
            normalization follows the pair's second AV. `filler` steps
            (tagged ("q", fn) for next-batch qkv — must finish here — or
            ("p", fn) for prev-batch proj) are spread between pairs to keep
            the PE dense; up to `carry` proj steps are left unconsumed and
            returned so the next (more starved) attention can use them."""
            filler = list(filler or [])
            supply = len(filler) - carry
            # fill slots: 5 S chunks/pair + 3 per AV pair (one between the
            # two heads' AVs so the PE has work while the DVE drains the
            # first head's PSUM banks) + 3 at the tail.
            total_slots = 5 * CA + 3 * CA + 3
            state = [0, 0]          # slots seen, steps consumed

            def fill(n, tail=False):
                state[0] += n
                target = min(supply - hold, min(eager, state[0])
                             + ((supply - eager) * state[0]) // total_slots)
                while state[1] < target and filler:
                    filler.pop(0)[1]()
                    state[1] += 1

            def fill_tail(n):
                fill(n)

            def flush_end():
                # all qkv steps must be emitted inside this attention (the
                # next batch's S matmuls precede them in PE program order);
                # leave up to `carry` proj steps for the next attention.
                keep = []
                n_proj = sum(1 for t, _ in filler if t == "p")
                for t, fn in filler:
                    if t == "q":
                        fn()
                    elif n_proj > carry:
                        fn()
                        n_proj -= 1
                    else:
                        keep.append((t, fn))
                return keep

            ptps = {}

            def do_av(j):
                ptp = ptps.pop(j)
                emit_av(v, 2 * j, ptp, at, j)
                fill(1)
                emit_av(v, 2 * j + 1, ptp, at, j)
                fill(2)

            for j in range(CA):
                ptps[j] = emit_pts_pair(qkt, j, fill=fill)
                if debug_taps and b == 0:
                    for kk in range(2):
                        nc.sync.dma_start(out=pt_tap[j, kk][:, 0:N - 128 * kk],
                                          in_=ptps[j][kk])
                if j >= av_lag:
                    do_av(j - av_lag)
            for j in range(CA - av_lag, CA):
                do_av(j)
            fill_tail(3)
            return flush_end()

        def proj_steps(b, at):
            """Proj work as a list of closures so it can be interleaved into
            the next batch's attention emission as PE filler."""
            steps = []
            for tt in range(5):
                for ch in range(2):
                    steps.append(("mm", b, at, tt, ch))
                    steps.append(("dma", b, tt, ch))
            return steps

        _ysb = {}

        def run_proj_step(step):
            kind = step[0]
            if kind == "mm":
                _, b, at, tt, ch = step
                tsz = TSZ[tt]
                to = 128 * tt
                if (b, tt) not in _ysb:
                    _ysb[(b, tt)] = y_pool.tile([128, C], bf16, tag="y",
                                                name=f"ysb{b}_{tt}")
                ysb = _ysb[(b, tt)]
                ps = ps_mm.tile([128, 512], f32, tag="mm", name="psp")
                for a in range(CA):
                    nc.tensor.matmul(
                        ps[:tsz, :384],
                        lhsT=at[:, a, to:to + tsz],
                        rhs=wp[:, a, 384 * ch:384 * (ch + 1)],
                        start=(a == 0), stop=(a == CA - 1),
                    )
                nc.vector.tensor_copy(
                    out=ysb[0:tsz, 384 * ch:384 * (ch + 1)],
                    in_=ps[:tsz, :384],
                )
            else:
                _, b, tt, ch = step
                tsz = TSZ[tt]
                to = 128 * tt
                ysb = _ysb[(b, tt)]
                if ch == 1:
                    _ysb.pop((b, tt))
                eng = nc.sync
                if b == BL - 1:
                    eng = (nc.sync, nc.gpsimd, nc.scalar)[(2 * tt + ch) % 3]
                eng.dma_start(
                    out=y_d[N * b + to:N * b + to + tsz,
                            384 * ch:384 * (ch + 1)],
                    in_=ysb[0:tsz, 384 * ch:384 * (ch + 1)],
                )

        def emit_proj(b, at):
            for step in proj_steps(b, at):
                run_proj_step(step)

        def interleave(a_list, b_list, ratio=2):
            """ratio a-items per b-item until either runs dry."""
            out = []
            ai = bi = 0
            while ai < len(a_list) or bi < len(b_list):
                for _ in range(ratio):
                    if ai < len(a_list):
                        out.append(a_list[ai])
                        ai += 1
                if bi < len(b_list):
                    out.append(b_list[bi])
                    bi += 1
            return out

        # software-pipelined batch loop: batch b+1's qkv and batch b-1's
        # proj are chopped into steps and threaded into batch b's attention
        # at every stall point, so the PE's in-order stream stays dense.
        xt, xt8 = emit_startup_loads()
        emit_weight_loads()
        # warm-up matmuls on a zeroed tile: fill the initial DMA-latency gap
        # with PE activity so the HAM clock gate reaches full speed before
        # the first real accumulation group, at no dependency cost.
        for _ in range(8):
            psw = ps_mm.tile([128, 512], f32, tag="mm", name="psw")
            nc.tensor.matmul(psw[:, :512], lhsT=warm[:, 0:128],
                             rhs=warm[:, 0:512], start=True, stop=True)
        qkt, v, _, fns = qkv_steps(0, xt, xt8)
        # prologue: Q/K projections for head pair 0 in ramp (a-major) order
        # so matmuls start as each x/wqk contraction tile lands; the rest of
        # batch 0's qkv is consumed eagerly (1 step/slot) inside attention 0.
        fns["ramp"](0)
        fns["ramp"](6)
        b0_rest = []

        def _q0(ot):
            for qc in (0, 288):
                b0_rest.append(("q", lambda ot=ot, qc=qc: fns["qk"](ot, qc)))

        def _v0(ch):
            for kk in range(5):
                b0_rest.append(("q", lambda kk=kk, ch=ch: fns["v"](kk, ch)))

        def _w0(n):
            def step():
                for _ in range(n):
                    psw = ps_mm.tile([128, 512], f32, tag="mm", name="psw")
                    nc.tensor.matmul(psw[:, :512], lhsT=warm[:, 0:128],
                                     rhs=warm[:, 0:512], start=True, stop=True)
            b0_rest.append(("q", step))

        _q0(1); _q0(7); _q0(2); _q0(8); _q0(3); _q0(9); _v0(0)
        _q0(4); _q0(10); _v0(1); _q0(5); _q0(11)
        pending = None
        carry_steps = []
        # proj steps carried from attention b to b+1: the last batch has no
        # next-batch qkv filler, so hold back some of the earlier proj work
        CARRY = {BL - 3: 12, BL - 2: 26}
        for b in range(BL):
            if debug_taps and b == 0:
                nc.sync.dma_start(out=qkt_tap[:, :, :], in_=qkt)
            filler = list(carry_steps)
            if pending is not None:
                filler += [("p", lambda s=s: run_proj_step(s))
                           for s in proj_steps(pending[0], pending[1])]
            nxt = None
            if b + 1 < BL:
                xt, xt8 = emit_xt(b + 1)
                qkt1, v1, qsteps, _ = qkv_steps(b + 1, xt, xt8)
                nxt = (qkt1, v1)
                filler = interleave([("q", s) for s in qsteps], filler,
                                    ratio=2)
            eager = 0
            if b == 0:
                filler = b0_rest + filler
                eager = len(b0_rest)
            at = at_pool.tile([128, CA, N], bf16, tag="at")
            carry_steps = emit_attention(b, qkt, v, at, filler=filler,
                                         carry=CARRY.get(b, 0), eager=eager,
                                         hold=6 if b == BL - 1 else 0,
                                         av_lag=2 if b == 0 else 1)
            pending = (b, at)
            if nxt is not None:
                qkt, v = nxt
        for _, fn in carry_steps:
            fn()
        emit_proj(pending[0], pending[1])

    nc.compile()
    return nc


def _get_program(have_qkb=False):
    key = ("nc", have_qkb)
    if key not in _prog_cache:
        _prog_cache[key] = _build_program(have_qkb)
    return _prog_cache[key]


def kernel(x, qkv_w, qkv_b, proj_w, proj_b):
    global LAST_EXEC_NS, LAST_TRACE_PATH
    from concourse.bass_utils import run_bass_kernel_spmd

    x = np.asarray(x, np.float32)
    qkv_w = np.asarray(qkv_w, np.float32)
    qkv_b = np.asarray(qkv_b, np.float32)
    proj_w = np.asarray(proj_w, np.float32)
    proj_b = np.asarray(proj_b, np.float32)

    perm = _block_perm()
    x_s = x[:, perm, :]

    have_qkb = bool(np.any(qkv_b[:2 * C]))
    use_fp8 = USE_FP8 and not have_qkb
    wqkT32 = np.ascontiguousarray(qkv_w[:2 * C].T)          # [C, 2C] f32
    wvT = np.ascontiguousarray(
        qkv_w[2 * C:].T.astype(BF16).reshape(CA, 128, C).transpose(1, 0, 2))
    wpT = np.ascontiguousarray(
        proj_w.T.astype(BF16).reshape(CA, 128, C).transpose(1, 0, 2))
    idx = np.arange(128)
    mask1 = (idx[:, None] // 16 <= idx[None, :] // 16).astype(BF16)
    maskT = np.ascontiguousarray(
        np.broadcast_to(mask1[:, None, :], (128, 2, 128)))

    shared = {"wvT": wvT, "wpT": wpT, "maskT": maskT}
    if use_fp8:
        # wqk8[p, r, i, o] = WS * wqkT[256p + 128i + r, o] in fp8 e4m3
        w8 = np.clip(wqkT32 * WS, -240, 240).astype(FP8)
        shared["wqk8"] = np.ascontiguousarray(
            w8.reshape(3, 2, 128, 2 * C).transpose(0, 2, 1, 3))
    else:
        shared["wqkT"] = np.ascontiguousarray(
            wqkT32.astype(BF16).reshape(CA, 128, 2, 6, 128)
            .transpose(2, 3, 1, 0, 4))
    if have_qkb:
        shared["qkb"] = np.ascontiguousarray(
            qkv_b[:2 * C].reshape(12, 128).T).astype(np.float32)
    in_maps = []
    for c in range(CORES):
        xb = x_s[BL * c:BL * (c + 1)]                        # [BL, N, C]
        xbT = np.ascontiguousarray(xb.transpose(0, 2, 1))    # [BL, C, N] f32
        m = {"xT": np.ascontiguousarray(
            xbT.astype(BF16).reshape(BL, CA, 128, N).transpose(0, 2, 1, 3))}
        m.update(shared)
        in_maps.append(m)

    nc = _get_program(have_qkb)
    res = None
    last_err = None
    for attempt in range(4):
        try:
            res = run_bass_kernel_spmd(nc, in_maps, core_ids=list(range(CORES)),
                                       trace=TRACE and attempt == 0)
            break
        except Exception as e:  # transient NRT/axon failures - retry
            last_err = e
            import time
            time.sleep(5 * (attempt + 1))
    if res is None:
        raise last_err
    LAST_EXEC_NS = res.exec_time_ns
    LAST_TRACE_PATH = (res.instructions_and_trace[1]
                       if res.instructions_and_trace else None)

    y_s = np.empty((B, N, C), np.float32)
    for c in range(CORES):
        y_s[BL * c:BL * (c + 1)] = res.results[c]["y"].astype(
            np.float32).reshape(BL, N, C)
    y = np.empty_like(y_s)
    y[:, perm, :] = y_s
    # v-bias and proj-bias contribute a constant per-channel vector to every
    # token: fold them in exactly here (attention rows sum to 1).
    tail = proj_b.astype(np.float64) + qkv_b[2 * C:].astype(np.float64) @ proj_w.T.astype(np.float64)
    if np.any(tail):
        y += tail.astype(np.float32)[None, None, :]
    return y

